# revision 56
# baseline (speedup 1.0000x reference)
"""Trainium2 Bass kernel for nn_Matching_layer (9x9 local correlation volume).

Computation (per batch element b):
    f1n = l2normalize(feature1[b]) over C;  f2n = l2normalize(feature2[b])
    out[b, dh*9+dw, y*64+x] = relu(<f2n[:, y+dh-4, x+dw-4], f1n[:, y, x]>)
    (out-of-range f2 positions contribute exactly 0)

Shapes: feature1/2 (16, 512, 64, 64) fp32 -> out (16, 81, 4096) fp32.

Strategy (8 NeuronCores, pure data parallelism, 2 images per core):
  * bf16 everywhere on-chip (cast during the input DMA, split into row-halves
    so normalization starts while the bottom half streams in).
  * f2 kept resident in SBUF as a y-padded plane [128c x 4chunk x (72*64+8)];
    x-borders are handled by masking the affected outputs.
  * Normalization without transposes: per 512-position strip, ACT squares ->
    PE one-hot-column ones-reduce routes strip s's sums to PSUM partition s,
    so sqrt/reciprocal run once per [4|8, 512] block.  f2's reciprocal norms
    are broadcast back over channels with a one-hot-row matmul and applied
    in-place with a single kc-broadcast DVE multiply per strip (two
    independent strip-halves, so the first Grams start early).  f1's norms
    are computed row-major (squares on DVE via strided views of the
    tile-major copy) and applied to the assembled output image, not per tile.
  * Main compute: per 16x8 position tile, PE computes the banded Gram tile
    G[128 pos, 384 window-pos] = f1_tile^T @ f2_window (4 K-chunks of 128);
    f1 is re-staged tile-major since matmul stationary operands must be
    single-free-dim.  Both images' Gram phases are emitted before the first
    extraction so the PE queue never sits behind a gather.
  * The 81 window dot products per position sit on a per-partition diagonal
    of G, which no on-chip engine can address.  G is relu'd (two tiles per
    PSUM read), cast to bf16, and written GB=8 tiles at a time to a DRAM
    scratch skew  addr = py*S_PY + px*S_PX + g*384 + q  chosen so the
    needed elements sit at  p*PSTEP + g*384 + dh*16 + dw.  One gather DMA
    per batch fetches contiguous [9x16] window blocks for all 8 tiles
    (one 288B descriptor per position per tile -- 9x fewer descriptors than
    gathering the 9x9 sub-blocks, 8x fewer DMAs).  G writes and gathers are
    spread across the SP/ACT/Pool DMA queues.
  * Densify+mask -> PE-transpose pairs -> pack into a bf16 [81, 4096] image,
    scale by rn1 strip-wise (one-hot-row broadcast matmuls), cast on the
    output DMA.
"""

import threading

import numpy as np

import concourse.bass as bass
import concourse.mybir as mybir
import concourse.tile as tile
from concourse.masks import make_identity
from concourse.vector_clock import ScopedClock

# ---------------------------------------------------------------- constants
B, C, H, W = 16, 512, 64, 64
PATCH, R = 9, 4
P2 = PATCH * PATCH            # 81
HWTOT = H * W                 # 4096
N_CORES = 8
B_LOC = B // N_CORES          # 2 images per core
NCH = C // 128                # 4 contraction chunks

BY, BX = 16, 8                # position tile (M = 128)
NTY, NTX = H // BY, W // BX   # 4 x 8 = 32 tiles per image
QY, QX = BY + 2 * R, BX + 2 * R   # 24 x 16 window block
Q = QY * QX                   # 384

# f2 plane: y-padded (R rows top/bottom), x handled by masks; 4-elem guards
PF = (H + 2 * R) * W + 2 * R          # 72*64 + 8 = 4616
PORIGIN = R                           # flat offset of plane (y=-4, x=0)
PINT = PORIGIN + R * W                # interior start = 4 + 256 = 260

# skewed DRAM layout for batched G (GB tiles per write/gather group):
#   write addr = py*S_PY + px*S_PX + (g*Q + q)   [3-dim affine, inner dense]
#   the needed element (p, g, dh, dw) then sits at p*PSTEP + g*Q + dh*QX + dw
GB = 8                                # tiles per batched G write DMA
S_PX = GB * Q + BX                    # 3080
S_PY = BX * (S_PX + 1) - QX           # 8*3081 - 16 = 24632
PSTEP = S_PX + 1                      # 3081
GSIZE = (BY - 1) * S_PY + (BX - 1) * S_PX + GB * Q + QX   # 394128
# (+QX: the wide row-block gather reads [g*Q, g*Q + 9*QX) per position, which
# for the last position/tile extends QX-PATCH elements past the written data)
# self-Gram diag layout (for sum-of-squares): addr = p*129 + p', diag at p*130
S1 = 129
G1SIZE = 127 * S1 + 128               # 16511

FP32 = mybir.dt.float32
BF16 = mybir.dt.bfloat16
AFT = mybir.ActivationFunctionType


# -------------------------------------------------- tile tail-drain workaround
# The walrus build in this container rejects a Drain instruction carrying more
# than one sync wait.  Split the tail waits into single-wait NOPs instead.
def _patched_drain_and_barrier(self, tick_clock, wait_clock):
    nc = self.nc
    probe = nc.sync.nop(nofuse=True)
    wait_clock.add_sem_waits(probe.ins, ScopedClock({None: tick_clock.global_clock}))
    waits = list(probe.ins.sync_info.on_wait)
    if len(waits) > 1:
        probe.ins.sync_info.on_wait = waits[:1]
        id2sem = {s.num: s for s in self.sems.allocated().values()}
        for w in waits[1:]:
            extra = nc.sync.nop(nofuse=True)
            extra.wait_op(id2sem[w.id], w.wait_value, "sem-ge")
    nc.sync.drain()
    nc.all_engine_barrier()
    popped = nc._tile_sem_poison_stack.pop()
    assert popped is self._sem_poison
    nc.clear_and_free_semaphores(list(self.sems.allocated().values()))
    nc.all_engine_barrier()


tile.TileContext._drain_and_barrier = _patched_drain_and_barrier


def _split_sync_waits(nc, max_waits=1):
    """The walrus build here only supports a limited number of sync waits per
    instruction.  Move excess waits onto engine-matched NOPs inserted just
    before the owning instruction (semantics preserved: the engine blocks on
    the nops first)."""
    import copy as _copy

    tmpl = None
    for f in nc.m.functions:
        for bb in f.blocks:
            for inst in bb.instructions:
                if inst.opcode == "NoOp":
                    tmpl = inst
                    break
            if tmpl is not None:
                break
        if tmpl is not None:
            break
    assert tmpl is not None, "no NoOp template found"
    uid = 0
    for f in nc.m.functions:
        for bb in f.blocks:
            new = []
            changed = False
            for inst in bb.instructions:
                si = inst.sync_info
                if si is not None and len(si.on_wait) > max_waits:
                    waits = list(si.on_wait)
                    extra, keep = waits[:-max_waits], waits[-max_waits:]
                    for i in range(0, len(extra), max_waits):
                        nop = _copy.deepcopy(tmpl)
                        nop.name = f"I-waitsplit-{uid}"
                        uid += 1
                        nop.engine = inst.engine
                        nop.sync_info = mybir.SyncInfo(
                            on_wait=extra[i : i + max_waits], on_update=[]
                        )
                        new.append(nop)
                    si.on_wait = keep
                    changed = True
                new.append(inst)
            if changed:
                bb.instructions = new


def _sub_ap(t, extra_offset, dims):
    """AP on t's tensor at t.offset + extra_offset with partition dim kept."""
    return bass.AP(
        tensor=t.tensor, offset=t.offset + extra_offset, ap=[list(t.ap[0])] + dims
    )


def _flat_ap(t, extra_offset, dims):
    """AP on a DRAM tile viewed as flat memory (no partition dim)."""
    return bass.AP(tensor=t.tensor, offset=t.offset + extra_offset, ap=dims)


def _rowsel_np():
    """Host-side constant: [8, 8*128] where [:, 128s:128(s+1)] is the one-hot
    row matrix selr[s][k, :] = (k == s)."""
    a = np.zeros((8, 8 * 128), np.float32)
    for s in range(8):
        a[s, 128 * s : 128 * (s + 1)] = 1.0
    return a


def build_matching_kernel(nc, f1, f2, mask0, mask7, out, repeat=1, mode="full"):
    """Emit Tile IR.  f1/f2: [B_LOC, C, H, W] fp32 DRAM; masks: [128, P2] fp32;
    out: [B_LOC, P2, H*W] fp32 DRAM.  repeat>1 re-runs the whole computation
    (for steady-state timing); pools rotate so the footprint is unchanged."""
    from contextlib import ExitStack

    rowsel = nc.dram_tensor(
        "rowsel", [8, 8 * 128], FP32, kind="ExternalInput"
    ).ap()

    with tile.TileContext(nc) as tc, ExitStack() as ctx:
        consts = ctx.enter_context(tc.tile_pool(name="consts", bufs=1))
        planes = ctx.enter_context(tc.tile_pool(name="planes", bufs=2))
        f1pool = ctx.enter_context(tc.tile_pool(name="f1pool", bufs=2))
        sqpool = ctx.enter_context(tc.tile_pool(name="sqpool", bufs=2))
        ldpool = ctx.enter_context(tc.tile_pool(name="ldpool", bufs=2))
        strip = ctx.enter_context(tc.tile_pool(name="strip", bufs=2))
        nrm = ctx.enter_context(tc.tile_pool(name="nrm", bufs=1))
        rn1pool = ctx.enter_context(tc.tile_pool(name="rn1", bufs=2))
        gsb_pool = ctx.enter_context(tc.tile_pool(name="gsb", bufs=2))
        exb_pool = ctx.enter_context(tc.tile_pool(name="exb", bufs=4))
        outpool = ctx.enter_context(tc.tile_pool(name="outimg", bufs=2))

        ps_ssq = ctx.enter_context(tc.tile_pool(name="ps_ssq", bufs=1, space="PSUM"))
        ps_bc = ctx.enter_context(tc.tile_pool(name="ps_bc", bufs=1, space="PSUM"))
        ps_g = ctx.enter_context(tc.tile_pool(name="ps_g", bufs=2, space="PSUM"))
        ps_tp = ctx.enter_context(tc.tile_pool(name="ps_tp", bufs=2, space="PSUM"))

        gdram = ctx.enter_context(
            tc.tile_pool(name="gdram", bufs=2 * NTY * NTX // GB, space="DRAM")
        )

        # ---------------- constants
        ident = consts.tile([128, 128], FP32)
        make_identity(nc, ident)
        identb = consts.tile([128, 128], BF16)
        nc.vector.tensor_copy(out=identb, in_=ident)
        ident1 = consts.tile([1, 1], FP32)
        nc.vector.memset(ident1, 1.0)
        ones_col = consts.tile([128, 1], BF16)
        nc.vector.memset(ones_col, 1.0)
        ones_row = consts.tile([1, 128], BF16)
        nc.vector.memset(ones_row, 1.0)
        m0 = consts.tile([128, P2], BF16)
        nc.gpsimd.dma_start(out=m0, in_=mask0[:, :])
        m7 = consts.tile([128, P2], BF16)
        nc.gpsimd.dma_start(out=m7, in_=mask7[:, :])
        eps = consts.tile([1, 1], FP32)
        nc.vector.memset(eps, 1e-6)
        eps128 = consts.tile([128, 1], FP32)
        nc.vector.memset(eps128, 1e-6)
        # one-hot column selectors: sel[s][:, m] = (m == s), for routing each
        # strip's ssq ones-reduce to PSUM partition s
        sel = []
        for s in range(8):
            st = consts.tile([128, 8], BF16, tag=f"sel{s}")
            nc.vector.memset(st, 0.0)
            nc.vector.memset(st[:, s : s + 1], 1.0)
            sel.append(st)
        # one-hot row selectors: selr[s][k, :] = (k == s), for broadcasting
        # row s of an [8, N] operand to all 128 output partitions (built by
        # PE-transposing sel[s]; per-partition memsets are not allowed)
        # one-hot row selectors: selr[s][k, :] = (k == s), for broadcasting
        # row s of an [8, N] operand to all 128 output partitions.  Loaded
        # from a host-provided constant (per-partition on-chip construction
        # is not possible: base partitions must be 0/32/64/96).
        selr_all = consts.tile([8, 8 * 128], BF16)
        nc.gpsimd.dma_start(out=selr_all, in_=rowsel[:, :])
        selr = [selr_all[:, 128 * s : 128 * (s + 1)] for s in range(8)]

        def emit_once():
            if mode in ("actbig", "actsmall", "dvebig"):
                n, fsz = (500, 2048) if mode == "actbig" else (500, 64)
                if mode == "dvebig":
                    n, fsz = 500, 2048
                probe = ldpool.tile([128, 2048], BF16, tag="probe")
                nc.vector.memset(probe, 0.5)
                for _ in range(n):
                    if mode == "dvebig":
                        nc.vector.tensor_mul(
                            probe[:, :fsz], probe[:, :fsz], probe[:, :fsz]
                        )
                    else:
                        nc.scalar.activation(
                            out=probe[:, :fsz], in_=probe[:, :fsz], func=AFT.Square
                        )
                oimg = outpool.tile([81, HWTOT], FP32)
                nc.vector.memset(oimg, 0.0)
                for img in range(B_LOC):
                    nc.sync.dma_start(out=out[img, :, :], in_=oimg)
                return
            if mode == "loadf32":
                # pure input-bandwidth probe: plain fp32 HWDGE loads, no cast
                for img in range(B_LOC):
                    for kc in range(NCH):
                        for src in (f1, f2):
                            sc = ldpool.tile([128, HWTOT // 2], FP32, tag="f32ld")
                            nc.sync.dma_start(
                                out=sc,
                                in_=src[img, kc * 128 : (kc + 1) * 128, : H // 2, :],
                            )
                            sc2 = ldpool.tile([128, HWTOT // 2], FP32, tag="f32ld")
                            nc.sync.dma_start(
                                out=sc2,
                                in_=src[img, kc * 128 : (kc + 1) * 128, H // 2 :, :],
                            )
                    oimg = outpool.tile([81, HWTOT], FP32)
                    nc.vector.memset(oimg, 0.0)
                    nc.sync.dma_start(out=out[img, :, :], in_=oimg)
                return
            plane = []
            f1p = []
            # ---------------- load both images up front (planes are double-buffered)
            # f1 is re-staged to a *tile-major* layout (128 contiguous elements
            # per 16x8 tile): the walrus BIR verifier only allows matmul
            # operands with a single free dimension, so the stationary operand
            # must be a plain slice.  Copies split across ACT and DVE.
            def load_img(img):
                pl = planes.tile([128, NCH, PF], BF16)
                fp = f1pool.tile([128, NCH, HWTOT], BF16)
                plane.append(pl)
                f1p.append(fp)
                hh = HWTOT // 2
                for kc in range(NCH):
                    nc.vector.memset(pl[:, kc, 0:PINT], 0.0)
                    nc.vector.memset(pl[:, kc, PINT + HWTOT : PF], 0.0)
                    fpv = fp[:, kc, :].rearrange(
                        "p (a c b d) -> p a c b d", a=NTY, c=NTX, b=BY, d=BX
                    )
                    # row-major (y x) -> tile-major (ty tx py px); ISA free APs
                    # are limited to 3 dims, so one copy per ty band.  Loads
                    # are split into row-halves (half-sized staging tiles,
                    # double-buffered) so normalization and the restage keep
                    # up with the stream instead of gating the next load.
                    for hf in range(2):
                        nc.gpsimd.dma_start(
                            out=pl[:, kc, PINT + hf * hh : PINT + (hf + 1) * hh],
                            in_=f2[
                                img, kc * 128 : (kc + 1) * 128,
                                hf * (H // 2) : (hf + 1) * (H // 2), :,
                            ],
                        )
                        fl = ldpool.tile([128, hh], BF16, tag="f1ld")
                        nc.gpsimd.dma_start(
                            out=fl,
                            in_=f1[
                                img, kc * 128 : (kc + 1) * 128,
                                hf * (H // 2) : (hf + 1) * (H // 2), :,
                            ],
                        )
                        flv = fl.rearrange(
                            "p (a b c d) -> p a c b d", a=2, b=BY, c=NTX, d=BX
                        )
                        for i, ty in enumerate(range(2 * hf, 2 * hf + 2)):
                            if ty % 2 == 0:
                                nc.scalar.copy(
                                    out=fpv[:, ty], in_=flv[:, i]
                                )
                            else:
                                nc.vector.tensor_copy(
                                    out=fpv[:, ty], in_=flv[:, i]
                                )

            if mode == "loadonly":
                for img in range(B_LOC):
                    load_img(img)
                    oimg = outpool.tile([81, HWTOT], FP32)
                    nc.vector.memset(oimg, 0.0)
                    nc.sync.dma_start(out=out[img, :, :], in_=oimg)
                return

            rn1bs = []

            # ---------------- normalization.  ssq per strip via ACT Square +
            # PE one-hot-column reduce: strip s's ones-reduce lands on PSUM
            # partition s, so all 8 strips accumulate into one [8, 512] tile
            # and Sqrt/reciprocal run once per image-feature, no transposes.
            def ssq_8strips(src_of_s):
                ssq8 = ps_ssq.tile([8, 512], FP32)
                for s in range(8):
                    sq4 = sqpool.tile([128, NCH, 512], BF16, tag="sq4")
                    nc.scalar.activation(
                        out=sq4, in_=src_of_s(s), func=AFT.Square
                    )
                    for kc in range(NCH):
                        nc.tensor.matmul(
                            ssq8,
                            lhsT=sel[s],
                            rhs=sq4[:, kc, :],
                            start=(s == 0 and kc == 0),
                            stop=(s == 7 and kc == NCH - 1),
                        )
                rn = nrm.tile([8, 512], FP32, tag="rn8")
                nc.scalar.activation(
                    out=rn, in_=ssq8, func=AFT.Sqrt, bias=eps128[0:8]
                )
                nc.vector.reciprocal(rn, rn)
                return rn

            def norm_f2(img):
                # two independent halves (strips 0-3 / 4-7) so the top
                # image-rows are normalized -- and their Grams can start --
                # while the bottom strips are still squaring
                pl = plane[img]
                for half in range(2):
                    ssq4 = ps_ssq.tile([4, 512], FP32, tag="ssq4")
                    for s in range(4 * half, 4 * half + 4):
                        sq4 = sqpool.tile([128, NCH, 512], BF16, tag="sq4")
                        nc.scalar.activation(
                            out=sq4,
                            in_=pl[:, :, PINT + 512 * s : PINT + 512 * (s + 1)],
                            func=AFT.Square,
                        )
                        for kc in range(NCH):
                            nc.tensor.matmul(
                                ssq4,
                                lhsT=sel[s][:, 4 * half : 4 * half + 4],
                                rhs=sq4[:, kc, :],
                                start=(s == 4 * half and kc == 0),
                                stop=(s == 4 * half + 3 and kc == NCH - 1),
                            )
                    rn2 = nrm.tile([4, 512], FP32, tag="rn4")
                    nc.scalar.activation(
                        out=rn2, in_=ssq4, func=AFT.Sqrt, bias=eps128[0:4]
                    )
                    nc.vector.reciprocal(rn2, rn2)
                    rnb = strip.tile([4, 512], BF16, tag="rnb4")
                    nc.vector.tensor_copy(out=rnb, in_=rn2)
                    for s in range(4 * half, 4 * half + 4):
                        off = PINT + 512 * s
                        bc = ps_bc.tile([128, 512], FP32)
                        nc.tensor.matmul(
                            bc, lhsT=selr[s % 4][0:4, :], rhs=rnb,
                            start=True, stop=True,
                        )
                        bcs = strip.tile([128, 512], BF16, tag="bcs")
                        nc.scalar.copy(out=bcs, in_=bc)
                        seg = pl[:, :, off : off + 512]
                        nc.vector.tensor_mul(
                            seg, seg, bcs.unsqueeze(1).broadcast_to(
                                (128, NCH, 512)
                            ),
                        )

            def norm_f1(img):
                # --- f1: squares on DVE (offloads ACT) reading *row-major*
                # strips through strided views of the tile-major layout, so
                # the reciprocal norms come out row-major [8, 512] and can be
                # applied to the assembled output image, not per tile.
                fp = f1p[img]
                ssq8 = ps_ssq.tile([8, 512], FP32, tag="ssq4")
                for s in range(8):
                    sq4 = sqpool.tile([128, NCH, 512], BF16, tag="sq4")
                    base = (s // 2) * NTX * BY * BX + (s % 2) * 8 * BX
                    for kc in range(NCH):
                        # row-major strip s = image rows 8s..8s+7: position
                        # (y, x) lives at tile t*128 + (y%16)*8 + (x%8)
                        src = _sub_ap(
                            fp[:, kc, :], base,
                            [[BX, 8], [BY * BX, NTX], [1, BX]],
                        )
                        dst = sq4[:, kc, :].rearrange(
                            "p (a b c) -> p a b c", a=8, b=NTX
                        )
                        nc.vector.tensor_mul(dst, src, src)
                    for kc in range(NCH):
                        nc.tensor.matmul(
                            ssq8,
                            lhsT=sel[s],
                            rhs=sq4[:, kc, :],
                            start=(s == 0 and kc == 0),
                            stop=(s == 7 and kc == NCH - 1),
                        )
                rn1f = nrm.tile([8, 512], FP32, tag="rn8")
                nc.scalar.activation(
                    out=rn1f, in_=ssq8, func=AFT.Sqrt, bias=eps128[0:8]
                )
                nc.vector.reciprocal(rn1f, rn1f)
                rn1b = strip.tile([8, 512], BF16, tag="rn1b")
                nc.vector.tensor_copy(out=rn1b, in_=rn1f)
                rn1bs.append(rn1b)

            if mode == "notiles":
                for img in range(B_LOC):
                    load_img(img)
                    norm_f2(img)
                    norm_f1(img)
                    oimg = outpool.tile([81, HWTOT], FP32)
                    nc.vector.memset(oimg, 0.0)
                    nc.sync.dma_start(out=out[img, :, :], in_=oimg)
                return

            # ---------------- main per-tile compute, phased across BOTH
            # images so the PE queue never sits behind a gather wait:
            # norm_f2(0), grams(0), norm_f2(1), grams(1), then per image:
            # norm_f1, gathers, densify+transpose+pack, rn1-scale, out DMA.
            wr_engines_by_img = [
                [nc.sync, nc.scalar, nc.gpsimd, nc.sync],
                [nc.sync, nc.scalar, nc.gpsimd, nc.sync],
            ]
            ga_engines_by_img = [
                [nc.sync, nc.gpsimd, nc.sync, nc.gpsimd],
                [nc.sync, nc.gpsimd, nc.sync, nc.gpsimd],
            ]

            def gram_batch(img, k, gds):
                pl, fp = plane[img], f1p[img]
                gsb = None
                gps2 = None
                for t in range(k * GB, (k + 1) * GB):
                    ty, tx = t // NTX, t % NTX
                    if t % 2 == 0:
                        # [2, 512]-padded so each tile's 384 columns stay
                        # inside one PSUM bank (matmul can't cross banks)
                        gps2 = ps_g.tile([128, 2, 512], FP32)
                    gps = gps2[:, t % 2, 0:Q]
                    for kc in range(NCH):
                        rhs = _sub_ap(
                            pl[:, kc, :],
                            ty * BY * W + tx * BX,
                            [[W, QY], [1, QX]],
                        )
                        nc.tensor.matmul(
                            gps,
                            lhsT=fp[:, kc, t * 128 : (t + 1) * 128],
                            rhs=rhs,
                            start=(kc == 0), stop=(kc == NCH - 1),
                        )
                    if mode == "noext":
                        continue
                    g = t % GB
                    if g == 0:
                        gsb = gsb_pool.tile([128, GB * Q], BF16)
                    if t % 2 == 1:
                        nc.vector.tensor_scalar_max(
                            gsb[:, (g - 1) * Q : (g + 1) * Q].rearrange(
                                "p (a b) -> p a b", a=2
                            ),
                            gps2[:, :, 0:Q], 0.0,
                        )
                    if g == GB - 1 and mode in ("full", "nogather"):
                        gd = gdram.tile([1, GSIZE], BF16)
                        wr_engines_by_img[img][k].dma_start(
                            out=_flat_ap(
                                gd, 0, [[S_PY, BY], [S_PX, BX], [1, GB * Q]]
                            ),
                            in_=gsb,
                        )
                        gds.append(gd)

            # fetch contiguous [9x16] window blocks per position, all GB
            # tiles of a batch in one DMA (one 288B descriptor per partition
            # per tile -- 9x fewer descriptors than the 9x9 sub-block, 8x
            # fewer DMAs); the unused columns 9..15 are never read.
            EXW = PATCH * QX  # 144

            def extract_batch(img, b, gds, oimg):
                exw = exb_pool.tile([128, GB, EXW], BF16)
                if mode == "full":
                    ga_engines_by_img[img][b].dma_start(
                        out=exw,
                        in_=_flat_ap(
                            gds[b], 0,
                            [[PSTEP, 128], [Q, GB], [1, EXW]],
                        ),
                    )
                else:
                    nc.vector.memset(exw, 0.25)
                tp2 = None
                for g in range(GB):
                    t = b * GB + g
                    ty, tx = t // NTX, t % NTX
                    exv = exw[:, g, :].rearrange(
                        "p (a b) -> p a b", b=QX
                    )[:, :, 0:PATCH]
                    # densify to [128, 81] (matmul/transpose operands must
                    # have a single free dim), fusing the border mask in
                    exb = exb_pool.tile([128, P2], BF16, tag="exb")
                    exbv = exb.rearrange("p (a b) -> p a b", b=PATCH)
                    if tx == 0:
                        nc.vector.tensor_mul(
                            exbv, exv,
                            m0.rearrange("p (a b) -> p a b", b=PATCH),
                        )
                    elif tx == NTX - 1:
                        nc.vector.tensor_mul(
                            exbv, exv,
                            m7.rearrange("p (a b) -> p a b", b=PATCH),
                        )
                    else:
                        nc.vector.tensor_copy(out=exbv, in_=exv)
                    if g % 2 == 0:
                        tp2 = ps_tp.tile([81, 2, 128], BF16)
                    nc.tensor.transpose(tp2[:, g % 2, :], exb, identb)
                    if g % 2 == 1:
                        # pack both tiles of the pair with one copy:
                        # out (py, pair, px), in (py, pair, px)
                        opair = bass.AP(
                            tensor=oimg.tensor,
                            offset=oimg.offset + ty * BY * W + (tx - 1) * BX,
                            ap=[list(oimg.ap[0]), [W, BY], [BX, 2], [1, BX]],
                        )
                        nc.vector.tensor_copy(
                            out=opair,
                            in_=bass.AP(
                                tensor=tp2.tensor,
                                offset=tp2.offset,
                                ap=[list(tp2.ap[0]), [BX, BY],
                                    [128, 2], [1, BX]],
                            ),
                        )

            def scales_out(img, oimg):
                # rn1 applied on the assembled image, strip-wise row-major
                if mode == "noext":
                    nc.vector.memset(oimg, 0.0)
                else:
                    for s in range(8):
                        bct = ps_bc.tile([128, 512], FP32, tag="bc")
                        nc.tensor.matmul(
                            bct[0:81, :], lhsT=selr[s][:, 0:P2],
                            rhs=rn1bs[img], start=True, stop=True,
                        )
                        bcs = strip.tile([81, 512], BF16, tag="bcs81")
                        nc.scalar.copy(out=bcs, in_=bct[0:81, :])
                        osl = oimg[:, 512 * s : 512 * (s + 1)]
                        nc.vector.tensor_mul(osl, osl, bcs)
                for h in range(2):
                    nc.gpsimd.dma_start(
                        out=out[img, :, h * (HWTOT // 2) : (h + 1) * (HWTOT // 2)],
                        in_=oimg[:, h * (HWTOT // 2) : (h + 1) * (HWTOT // 2)],
                    )

            NB_ = NTY * NTX // GB  # 4 batches per image
            gds0, gds1 = [], []
            # img1's loads are emitted after img0's normalization so img0's
            # squares don't queue behind img1's restage copies on ACT/DVE
            load_img(0)
            norm_f2(0)
            load_img(1)
            for k in range(NB_):
                gram_batch(0, k, gds0)
            norm_f2(1)
            oimg0 = outpool.tile([81, HWTOT], BF16, tag="oimg")
            oimg1 = outpool.tile([81, HWTOT], BF16, tag="oimg")
            # interleave img1's Gram batches with img0's extraction batches:
            # img0's gathers/densify/packs overlap img1's matmuls instead of
            # queueing behind all of img1's relu work.  The f1 normalizations
            # and the finished images' scale/out phases are likewise spread
            # into the pipeline so no engine queue holds later batches back.
            for k in range(NB_):
                gram_batch(1, k, gds1)
                if mode != "noext":
                    extract_batch(0, k, gds0, oimg0)
            norm_f1(0)
            for k in range(NB_):
                if mode != "noext":
                    extract_batch(1, k, gds1, oimg1)
                if k == 0:
                    scales_out(0, oimg0)
                elif k == 1:
                    norm_f1(1)
            scales_out(1, oimg1)

        for _rep in range(repeat):
            emit_once()
    return nc


# ---------------------------------------------------------------- host side
def _edge_masks():
    p = np.arange(128)
    d = np.arange(P2)
    px = (p % BX)[:, None]
    dw = (d % PATCH)[None, :]
    # tx = 0:      x_img = px + dw - R >= 0             <=>  px + dw >= R
    # tx = NTX-1:  x_img = (NTX-1)*BX + px + dw - R < W <=>  px + dw < BX + R
    m0 = (px + dw >= R).astype(np.float32)
    m7 = (px + dw < BX + R).astype(np.float32)
    return m0, m7


_cache = threading.local()


def _get_compiled():
    if getattr(_cache, "nc", None) is None:
        nc = bass.Bass()
        f1 = nc.dram_tensor("feature1", [B_LOC, C, H, W], FP32, kind="ExternalInput")
        f2 = nc.dram_tensor("feature2", [B_LOC, C, H, W], FP32, kind="ExternalInput")
        mask0 = nc.dram_tensor("mask0", [128, P2], FP32, kind="ExternalInput")
        mask7 = nc.dram_tensor("mask7", [128, P2], FP32, kind="ExternalInput")
        out = nc.dram_tensor("out", [B_LOC, P2, HWTOT], FP32, kind="ExternalOutput")
        build_matching_kernel(nc, f1.ap(), f2.ap(), mask0.ap(), mask7.ap(), out.ap())
        _split_sync_waits(nc, max_waits=1)
        _cache.nc = nc
    return _cache.nc


def kernel(feature1: np.ndarray, feature2: np.ndarray) -> np.ndarray:
    from concourse.bass_utils import run_bass_kernel_spmd

    feature1 = np.ascontiguousarray(feature1, dtype=np.float32)
    feature2 = np.ascontiguousarray(feature2, dtype=np.float32)
    nc = _get_compiled()
    m0, m7 = _edge_masks()
    rsel = _rowsel_np()
    in_maps = []
    for c in range(N_CORES):
        sl = slice(c * B_LOC, (c + 1) * B_LOC)
        in_maps.append(
            {
                "feature1": feature1[sl],
                "feature2": feature2[sl],
                "mask0": m0,
                "mask7": m7,
                "rowsel": rsel,
            }
        )
    res = run_bass_kernel_spmd(nc, in_maps, core_ids=list(range(N_CORES)))
    out = np.concatenate([res.results[c]["out"] for c in range(N_CORES)], axis=0)
    return out.reshape(B, P2, HWTOT)



# revision 59
# speedup vs baseline: 1.0036x; 1.0036x over previous
"""Trainium2 Bass kernel for nn_Matching_layer (9x9 local correlation volume).

Computation (per batch element b):
    f1n = l2normalize(feature1[b]) over C;  f2n = l2normalize(feature2[b])
    out[b, dh*9+dw, y*64+x] = relu(<f2n[:, y+dh-4, x+dw-4], f1n[:, y, x]>)
    (out-of-range f2 positions contribute exactly 0)

Shapes: feature1/2 (16, 512, 64, 64) fp32 -> out (16, 81, 4096) fp32.

Strategy (8 NeuronCores, pure data parallelism, 2 images per core):
  * bf16 everywhere on-chip (cast during the input DMA, split into row-halves
    so normalization starts while the bottom half streams in).
  * f2 kept resident in SBUF as a y-padded plane [128c x 4chunk x (72*64+8)];
    x-borders are handled by masking the affected outputs.
  * Normalization without transposes: per 512-position strip, ACT squares ->
    PE one-hot-column ones-reduce routes strip s's sums to PSUM partition s,
    so sqrt/reciprocal run once per [4|8, 512] block.  f2's reciprocal norms
    are broadcast back over channels with a one-hot-row matmul and applied
    in-place with a single kc-broadcast DVE multiply per strip (two
    independent strip-halves, so the first Grams start early).  f1's norms
    are computed row-major (squares on DVE via strided views of the
    tile-major copy) and applied to the assembled output image, not per tile.
  * Main compute: per 16x8 position tile, PE computes the banded Gram tile
    G[128 pos, 384 window-pos] = f1_tile^T @ f2_window (4 K-chunks of 128);
    f1 is re-staged tile-major since matmul stationary operands must be
    single-free-dim.  Both images' Gram phases are emitted before the first
    extraction so the PE queue never sits behind a gather.
  * The 81 window dot products per position sit on a per-partition diagonal
    of G, which no on-chip engine can address.  G is relu'd (two tiles per
    PSUM read), cast to bf16, and written GB=8 tiles at a time to a DRAM
    scratch skew  addr = py*S_PY + px*S_PX + g*384 + q  chosen so the
    needed elements sit at  p*PSTEP + g*384 + dh*16 + dw.  One gather DMA
    per batch fetches contiguous [9x16] window blocks for all 8 tiles
    (one 288B descriptor per position per tile -- 9x fewer descriptors than
    gathering the 9x9 sub-blocks, 8x fewer DMAs).  G writes and gathers are
    spread across the SP/ACT/Pool DMA queues.
  * Densify+mask -> PE-transpose pairs -> pack into a bf16 [81, 4096] image,
    scale by rn1 strip-wise (one-hot-row broadcast matmuls), cast on the
    output DMA.
"""

import threading

import numpy as np

import concourse.bass as bass
import concourse.mybir as mybir
import concourse.tile as tile
from concourse.masks import make_identity
from concourse.vector_clock import ScopedClock

# ---------------------------------------------------------------- constants
B, C, H, W = 16, 512, 64, 64
PATCH, R = 9, 4
P2 = PATCH * PATCH            # 81
HWTOT = H * W                 # 4096
N_CORES = 8
B_LOC = B // N_CORES          # 2 images per core
NCH = C // 128                # 4 contraction chunks

BY, BX = 16, 8                # position tile (M = 128)
NTY, NTX = H // BY, W // BX   # 4 x 8 = 32 tiles per image
QY, QX = BY + 2 * R, BX + 2 * R   # 24 x 16 window block
Q = QY * QX                   # 384

# f2 plane: y-padded (R rows top/bottom), x handled by masks; 4-elem guards
PF = (H + 2 * R) * W + 2 * R          # 72*64 + 8 = 4616
PORIGIN = R                           # flat offset of plane (y=-4, x=0)
PINT = PORIGIN + R * W                # interior start = 4 + 256 = 260

# skewed DRAM layout for batched G (GB tiles per write/gather group):
#   write addr = py*S_PY + px*S_PX + (g*Q + q)   [3-dim affine, inner dense]
#   the needed element (p, g, dh, dw) then sits at p*PSTEP + g*Q + dh*QX + dw
GB = 8                                # tiles per batched G write DMA
S_PX = GB * Q + BX                    # 3080
S_PY = BX * (S_PX + 1) - QX           # 8*3081 - 16 = 24632
PSTEP = S_PX + 1                      # 3081
GSIZE = (BY - 1) * S_PY + (BX - 1) * S_PX + GB * Q + QX   # 394128
# (+QX: the wide row-block gather reads [g*Q, g*Q + 9*QX) per position, which
# for the last position/tile extends QX-PATCH elements past the written data)
# self-Gram diag layout (for sum-of-squares): addr = p*129 + p', diag at p*130
S1 = 129
G1SIZE = 127 * S1 + 128               # 16511

FP32 = mybir.dt.float32
BF16 = mybir.dt.bfloat16
AFT = mybir.ActivationFunctionType


# -------------------------------------------------- tile tail-drain workaround
# The walrus build in this container rejects a Drain instruction carrying more
# than one sync wait.  Split the tail waits into single-wait NOPs instead.
def _patched_drain_and_barrier(self, tick_clock, wait_clock):
    nc = self.nc
    probe = nc.sync.nop(nofuse=True)
    wait_clock.add_sem_waits(probe.ins, ScopedClock({None: tick_clock.global_clock}))
    waits = list(probe.ins.sync_info.on_wait)
    if len(waits) > 1:
        probe.ins.sync_info.on_wait = waits[:1]
        id2sem = {s.num: s for s in self.sems.allocated().values()}
        for w in waits[1:]:
            extra = nc.sync.nop(nofuse=True)
            extra.wait_op(id2sem[w.id], w.wait_value, "sem-ge")
    nc.sync.drain()
    nc.all_engine_barrier()
    popped = nc._tile_sem_poison_stack.pop()
    assert popped is self._sem_poison
    nc.clear_and_free_semaphores(list(self.sems.allocated().values()))
    nc.all_engine_barrier()


tile.TileContext._drain_and_barrier = _patched_drain_and_barrier


def _split_sync_waits(nc, max_waits=1):
    """The walrus build here only supports a limited number of sync waits per
    instruction.  Move excess waits onto engine-matched NOPs inserted just
    before the owning instruction (semantics preserved: the engine blocks on
    the nops first)."""
    import copy as _copy

    tmpl = None
    for f in nc.m.functions:
        for bb in f.blocks:
            for inst in bb.instructions:
                if inst.opcode == "NoOp":
                    tmpl = inst
                    break
            if tmpl is not None:
                break
        if tmpl is not None:
            break
    assert tmpl is not None, "no NoOp template found"
    uid = 0
    for f in nc.m.functions:
        for bb in f.blocks:
            new = []
            changed = False
            for inst in bb.instructions:
                si = inst.sync_info
                if si is not None and len(si.on_wait) > max_waits:
                    waits = list(si.on_wait)
                    extra, keep = waits[:-max_waits], waits[-max_waits:]
                    for i in range(0, len(extra), max_waits):
                        nop = _copy.deepcopy(tmpl)
                        nop.name = f"I-waitsplit-{uid}"
                        uid += 1
                        nop.engine = inst.engine
                        nop.sync_info = mybir.SyncInfo(
                            on_wait=extra[i : i + max_waits], on_update=[]
                        )
                        new.append(nop)
                    si.on_wait = keep
                    changed = True
                new.append(inst)
            if changed:
                bb.instructions = new


def _sub_ap(t, extra_offset, dims):
    """AP on t's tensor at t.offset + extra_offset with partition dim kept."""
    return bass.AP(
        tensor=t.tensor, offset=t.offset + extra_offset, ap=[list(t.ap[0])] + dims
    )


def _flat_ap(t, extra_offset, dims):
    """AP on a DRAM tile viewed as flat memory (no partition dim)."""
    return bass.AP(tensor=t.tensor, offset=t.offset + extra_offset, ap=dims)


def _rowsel_np():
    """Host-side constant: [8, 8*128] where [:, 128s:128(s+1)] is the one-hot
    row matrix selr[s][k, :] = (k == s)."""
    a = np.zeros((8, 8 * 128), np.float32)
    for s in range(8):
        a[s, 128 * s : 128 * (s + 1)] = 1.0
    return a


def build_matching_kernel(nc, f1, f2, mask0, mask7, out, repeat=1, mode="full"):
    """Emit Tile IR.  f1/f2: [B_LOC, C, H, W] fp32 DRAM; masks: [128, P2] fp32;
    out: [B_LOC, P2, H*W] fp32 DRAM.  repeat>1 re-runs the whole computation
    (for steady-state timing); pools rotate so the footprint is unchanged."""
    from contextlib import ExitStack

    rowsel = nc.dram_tensor(
        "rowsel", [8, 8 * 128], FP32, kind="ExternalInput"
    ).ap()

    with tile.TileContext(nc) as tc, ExitStack() as ctx:
        consts = ctx.enter_context(tc.tile_pool(name="consts", bufs=1))
        planes = ctx.enter_context(tc.tile_pool(name="planes", bufs=2))
        f1pool = ctx.enter_context(tc.tile_pool(name="f1pool", bufs=2))
        sqpool = ctx.enter_context(tc.tile_pool(name="sqpool", bufs=2))
        ldpool = ctx.enter_context(tc.tile_pool(name="ldpool", bufs=2))
        strip = ctx.enter_context(tc.tile_pool(name="strip", bufs=2))
        nrm = ctx.enter_context(tc.tile_pool(name="nrm", bufs=1))
        rn1pool = ctx.enter_context(tc.tile_pool(name="rn1", bufs=2))
        gsb_pool = ctx.enter_context(tc.tile_pool(name="gsb", bufs=2))
        exb_pool = ctx.enter_context(tc.tile_pool(name="exb", bufs=4))
        outpool = ctx.enter_context(tc.tile_pool(name="outimg", bufs=2))

        ps_ssq = ctx.enter_context(tc.tile_pool(name="ps_ssq", bufs=1, space="PSUM"))
        ps_bc = ctx.enter_context(tc.tile_pool(name="ps_bc", bufs=1, space="PSUM"))
        ps_g = ctx.enter_context(tc.tile_pool(name="ps_g", bufs=2, space="PSUM"))
        ps_tp = ctx.enter_context(tc.tile_pool(name="ps_tp", bufs=2, space="PSUM"))

        gdram = ctx.enter_context(
            tc.tile_pool(name="gdram", bufs=2 * NTY * NTX // GB, space="DRAM")
        )

        # ---------------- constants
        ident = consts.tile([128, 128], FP32)
        make_identity(nc, ident)
        identb = consts.tile([128, 128], BF16)
        nc.vector.tensor_copy(out=identb, in_=ident)
        ident1 = consts.tile([1, 1], FP32)
        nc.vector.memset(ident1, 1.0)
        ones_col = consts.tile([128, 1], BF16)
        nc.vector.memset(ones_col, 1.0)
        ones_row = consts.tile([1, 128], BF16)
        nc.vector.memset(ones_row, 1.0)
        m0 = consts.tile([128, P2], BF16)
        nc.gpsimd.dma_start(out=m0, in_=mask0[:, :])
        m7 = consts.tile([128, P2], BF16)
        nc.gpsimd.dma_start(out=m7, in_=mask7[:, :])
        eps = consts.tile([1, 1], FP32)
        nc.vector.memset(eps, 1e-6)
        eps128 = consts.tile([128, 1], FP32)
        nc.vector.memset(eps128, 1e-6)
        # one-hot column selectors: sel[s][:, m] = (m == s), for routing each
        # strip's ssq ones-reduce to PSUM partition s
        sel = []
        for s in range(8):
            st = consts.tile([128, 8], BF16, tag=f"sel{s}")
            nc.vector.memset(st, 0.0)
            nc.vector.memset(st[:, s : s + 1], 1.0)
            sel.append(st)
        # one-hot row selectors: selr[s][k, :] = (k == s), for broadcasting
        # row s of an [8, N] operand to all 128 output partitions (built by
        # PE-transposing sel[s]; per-partition memsets are not allowed)
        # one-hot row selectors: selr[s][k, :] = (k == s), for broadcasting
        # row s of an [8, N] operand to all 128 output partitions.  Loaded
        # from a host-provided constant (per-partition on-chip construction
        # is not possible: base partitions must be 0/32/64/96).
        selr_all = consts.tile([8, 8 * 128], BF16)
        nc.gpsimd.dma_start(out=selr_all, in_=rowsel[:, :])
        selr = [selr_all[:, 128 * s : 128 * (s + 1)] for s in range(8)]

        def emit_once():
            if mode in ("actbig", "actsmall", "dvebig"):
                n, fsz = (500, 2048) if mode == "actbig" else (500, 64)
                if mode == "dvebig":
                    n, fsz = 500, 2048
                probe = ldpool.tile([128, 2048], BF16, tag="probe")
                nc.vector.memset(probe, 0.5)
                for _ in range(n):
                    if mode == "dvebig":
                        nc.vector.tensor_mul(
                            probe[:, :fsz], probe[:, :fsz], probe[:, :fsz]
                        )
                    else:
                        nc.scalar.activation(
                            out=probe[:, :fsz], in_=probe[:, :fsz], func=AFT.Square
                        )
                oimg = outpool.tile([81, HWTOT], FP32)
                nc.vector.memset(oimg, 0.0)
                for img in range(B_LOC):
                    nc.sync.dma_start(out=out[img, :, :], in_=oimg)
                return
            if mode == "loadf32":
                # pure input-bandwidth probe: plain fp32 HWDGE loads, no cast
                for img in range(B_LOC):
                    for kc in range(NCH):
                        for src in (f1, f2):
                            sc = ldpool.tile([128, HWTOT // 2], FP32, tag="f32ld")
                            nc.sync.dma_start(
                                out=sc,
                                in_=src[img, kc * 128 : (kc + 1) * 128, : H // 2, :],
                            )
                            sc2 = ldpool.tile([128, HWTOT // 2], FP32, tag="f32ld")
                            nc.sync.dma_start(
                                out=sc2,
                                in_=src[img, kc * 128 : (kc + 1) * 128, H // 2 :, :],
                            )
                    oimg = outpool.tile([81, HWTOT], FP32)
                    nc.vector.memset(oimg, 0.0)
                    nc.sync.dma_start(out=out[img, :, :], in_=oimg)
                return
            plane = []
            f1p = []
            # ---------------- load both images up front (planes are double-buffered)
            # f1 is re-staged to a *tile-major* layout (128 contiguous elements
            # per 16x8 tile): the walrus BIR verifier only allows matmul
            # operands with a single free dimension, so the stationary operand
            # must be a plain slice.  Copies split across ACT and DVE.
            def load_img(img):
                pl = planes.tile([128, NCH, PF], BF16)
                fp = f1pool.tile([128, NCH, HWTOT], BF16)
                plane.append(pl)
                f1p.append(fp)
                hh = HWTOT // 2
                for kc in range(NCH):
                    nc.vector.memset(pl[:, kc, 0:PINT], 0.0)
                    nc.vector.memset(pl[:, kc, PINT + HWTOT : PF], 0.0)
                # loads split into row-halves; within each half all four f2
                # chunks stream FIRST (the normalization squares read across
                # all chunks, so this unblocks them ~7us earlier), then f1
                # chunks each followed by their tile-major restage copies
                # (half-sized staging tiles, double-buffered)
                for hf in range(2):
                    for kc in range(NCH):
                        nc.gpsimd.dma_start(
                            out=pl[:, kc, PINT + hf * hh : PINT + (hf + 1) * hh],
                            in_=f2[
                                img, kc * 128 : (kc + 1) * 128,
                                hf * (H // 2) : (hf + 1) * (H // 2), :,
                            ],
                        )
                    for kc in range(NCH):
                        fpv = fp[:, kc, :].rearrange(
                            "p (a c b d) -> p a c b d", a=NTY, c=NTX, b=BY, d=BX
                        )
                        fl = ldpool.tile([128, hh], BF16, tag="f1ld")
                        nc.gpsimd.dma_start(
                            out=fl,
                            in_=f1[
                                img, kc * 128 : (kc + 1) * 128,
                                hf * (H // 2) : (hf + 1) * (H // 2), :,
                            ],
                        )
                        flv = fl.rearrange(
                            "p (a b c d) -> p a c b d", a=2, b=BY, c=NTX, d=BX
                        )
                        for i, ty in enumerate(range(2 * hf, 2 * hf + 2)):
                            if ty % 2 == 0:
                                nc.scalar.copy(
                                    out=fpv[:, ty], in_=flv[:, i]
                                )
                            else:
                                nc.vector.tensor_copy(
                                    out=fpv[:, ty], in_=flv[:, i]
                                )

            if mode == "loadonly":
                for img in range(B_LOC):
                    load_img(img)
                    oimg = outpool.tile([81, HWTOT], FP32)
                    nc.vector.memset(oimg, 0.0)
                    nc.sync.dma_start(out=out[img, :, :], in_=oimg)
                return

            rn1bs = []

            # ---------------- normalization.  ssq per strip via ACT Square +
            # PE one-hot-column reduce: strip s's ones-reduce lands on PSUM
            # partition s, so all 8 strips accumulate into one [8, 512] tile
            # and Sqrt/reciprocal run once per image-feature, no transposes.
            def ssq_8strips(src_of_s):
                ssq8 = ps_ssq.tile([8, 512], FP32)
                for s in range(8):
                    sq4 = sqpool.tile([128, NCH, 512], BF16, tag="sq4")
                    nc.scalar.activation(
                        out=sq4, in_=src_of_s(s), func=AFT.Square
                    )
                    for kc in range(NCH):
                        nc.tensor.matmul(
                            ssq8,
                            lhsT=sel[s],
                            rhs=sq4[:, kc, :],
                            start=(s == 0 and kc == 0),
                            stop=(s == 7 and kc == NCH - 1),
                        )
                rn = nrm.tile([8, 512], FP32, tag="rn8")
                nc.scalar.activation(
                    out=rn, in_=ssq8, func=AFT.Sqrt, bias=eps128[0:8]
                )
                nc.vector.reciprocal(rn, rn)
                return rn

            def norm_f2(img):
                # two independent halves (strips 0-3 / 4-7) so the top
                # image-rows are normalized -- and their Grams can start --
                # while the bottom strips are still squaring
                pl = plane[img]
                for half in range(2):
                    ssq4 = ps_ssq.tile([4, 512], FP32, tag="ssq4")
                    for s in range(4 * half, 4 * half + 4):
                        sq4 = sqpool.tile([128, NCH, 512], BF16, tag="sq4")
                        nc.scalar.activation(
                            out=sq4,
                            in_=pl[:, :, PINT + 512 * s : PINT + 512 * (s + 1)],
                            func=AFT.Square,
                        )
                        for kc in range(NCH):
                            nc.tensor.matmul(
                                ssq4,
                                lhsT=sel[s][:, 4 * half : 4 * half + 4],
                                rhs=sq4[:, kc, :],
                                start=(s == 4 * half and kc == 0),
                                stop=(s == 4 * half + 3 and kc == NCH - 1),
                            )
                    rn2 = nrm.tile([4, 512], FP32, tag="rn4")
                    nc.scalar.activation(
                        out=rn2, in_=ssq4, func=AFT.Sqrt, bias=eps128[0:4]
                    )
                    nc.vector.reciprocal(rn2, rn2)
                    rnb = strip.tile([4, 512], BF16, tag="rnb4")
                    nc.vector.tensor_copy(out=rnb, in_=rn2)
                    for s in range(4 * half, 4 * half + 4):
                        off = PINT + 512 * s
                        bc = ps_bc.tile([128, 512], FP32)
                        nc.tensor.matmul(
                            bc, lhsT=selr[s % 4][0:4, :], rhs=rnb,
                            start=True, stop=True,
                        )
                        bcs = strip.tile([128, 512], BF16, tag="bcs")
                        nc.scalar.copy(out=bcs, in_=bc)
                        seg = pl[:, :, off : off + 512]
                        nc.vector.tensor_mul(
                            seg, seg, bcs.unsqueeze(1).broadcast_to(
                                (128, NCH, 512)
                            ),
                        )

            def norm_f1(img):
                # --- f1: squares on DVE (offloads ACT) reading *row-major*
                # strips through strided views of the tile-major layout, so
                # the reciprocal norms come out row-major [8, 512] and can be
                # applied to the assembled output image, not per tile.
                fp = f1p[img]
                ssq8 = ps_ssq.tile([8, 512], FP32, tag="ssq4")
                for s in range(8):
                    sq4 = sqpool.tile([128, NCH, 512], BF16, tag="sq4")
                    base = (s // 2) * NTX * BY * BX + (s % 2) * 8 * BX
                    for kc in range(NCH):
                        # row-major strip s = image rows 8s..8s+7: position
                        # (y, x) lives at tile t*128 + (y%16)*8 + (x%8)
                        src = _sub_ap(
                            fp[:, kc, :], base,
                            [[BX, 8], [BY * BX, NTX], [1, BX]],
                        )
                        dst = sq4[:, kc, :].rearrange(
                            "p (a b c) -> p a b c", a=8, b=NTX
                        )
                        nc.vector.tensor_mul(dst, src, src)
                    for kc in range(NCH):
                        nc.tensor.matmul(
                            ssq8,
                            lhsT=sel[s],
                            rhs=sq4[:, kc, :],
                            start=(s == 0 and kc == 0),
                            stop=(s == 7 and kc == NCH - 1),
                        )
                rn1f = nrm.tile([8, 512], FP32, tag="rn8")
                nc.scalar.activation(
                    out=rn1f, in_=ssq8, func=AFT.Sqrt, bias=eps128[0:8]
                )
                nc.vector.reciprocal(rn1f, rn1f)
                rn1b = strip.tile([8, 512], BF16, tag="rn1b")
                nc.vector.tensor_copy(out=rn1b, in_=rn1f)
                rn1bs.append(rn1b)

            if mode == "notiles":
                for img in range(B_LOC):
                    load_img(img)
                    norm_f2(img)
                    norm_f1(img)
                    oimg = outpool.tile([81, HWTOT], FP32)
                    nc.vector.memset(oimg, 0.0)
                    nc.sync.dma_start(out=out[img, :, :], in_=oimg)
                return

            # ---------------- main per-tile compute, phased across BOTH
            # images so the PE queue never sits behind a gather wait:
            # norm_f2(0), grams(0), norm_f2(1), grams(1), then per image:
            # norm_f1, gathers, densify+transpose+pack, rn1-scale, out DMA.
            wr_engines_by_img = [
                [nc.sync, nc.scalar, nc.gpsimd, nc.sync],
                [nc.sync, nc.scalar, nc.gpsimd, nc.sync],
            ]
            ga_engines_by_img = [
                [nc.sync, nc.gpsimd, nc.sync, nc.gpsimd],
                [nc.sync, nc.gpsimd, nc.sync, nc.gpsimd],
            ]

            def gram_batch(img, k, gds):
                pl, fp = plane[img], f1p[img]
                gsb = None
                gps2 = None
                for t in range(k * GB, (k + 1) * GB):
                    ty, tx = t // NTX, t % NTX
                    if t % 2 == 0:
                        # [2, 512]-padded so each tile's 384 columns stay
                        # inside one PSUM bank (matmul can't cross banks)
                        gps2 = ps_g.tile([128, 2, 512], FP32)
                    gps = gps2[:, t % 2, 0:Q]
                    for kc in range(NCH):
                        rhs = _sub_ap(
                            pl[:, kc, :],
                            ty * BY * W + tx * BX,
                            [[W, QY], [1, QX]],
                        )
                        nc.tensor.matmul(
                            gps,
                            lhsT=fp[:, kc, t * 128 : (t + 1) * 128],
                            rhs=rhs,
                            start=(kc == 0), stop=(kc == NCH - 1),
                        )
                    if mode == "noext":
                        continue
                    g = t % GB
                    if g == 0:
                        gsb = gsb_pool.tile([128, GB * Q], BF16)
                    if t % 2 == 1:
                        nc.vector.tensor_scalar_max(
                            gsb[:, (g - 1) * Q : (g + 1) * Q].rearrange(
                                "p (a b) -> p a b", a=2
                            ),
                            gps2[:, :, 0:Q], 0.0,
                        )
                    if g == GB - 1 and mode in ("full", "nogather"):
                        gd = gdram.tile([1, GSIZE], BF16)
                        wr_engines_by_img[img][k].dma_start(
                            out=_flat_ap(
                                gd, 0, [[S_PY, BY], [S_PX, BX], [1, GB * Q]]
                            ),
                            in_=gsb,
                        )
                        gds.append(gd)

            # fetch contiguous [9x16] window blocks per position, all GB
            # tiles of a batch in one DMA (one 288B descriptor per partition
            # per tile -- 9x fewer descriptors than the 9x9 sub-block, 8x
            # fewer DMAs); the unused columns 9..15 are never read.
            EXW = PATCH * QX  # 144

            def extract_batch(img, b, gds, oimg):
                exw = exb_pool.tile([128, GB, EXW], BF16)
                if mode == "full":
                    ga_engines_by_img[img][b].dma_start(
                        out=exw,
                        in_=_flat_ap(
                            gds[b], 0,
                            [[PSTEP, 128], [Q, GB], [1, EXW]],
                        ),
                    )
                else:
                    nc.vector.memset(exw, 0.25)
                tp2 = None
                for g in range(GB):
                    t = b * GB + g
                    ty, tx = t // NTX, t % NTX
                    exv = exw[:, g, :].rearrange(
                        "p (a b) -> p a b", b=QX
                    )[:, :, 0:PATCH]
                    # densify to [128, 81] (matmul/transpose operands must
                    # have a single free dim), fusing the border mask in
                    exb = exb_pool.tile([128, P2], BF16, tag="exb")
                    exbv = exb.rearrange("p (a b) -> p a b", b=PATCH)
                    if tx == 0:
                        nc.vector.tensor_mul(
                            exbv, exv,
                            m0.rearrange("p (a b) -> p a b", b=PATCH),
                        )
                    elif tx == NTX - 1:
                        nc.vector.tensor_mul(
                            exbv, exv,
                            m7.rearrange("p (a b) -> p a b", b=PATCH),
                        )
                    else:
                        nc.vector.tensor_copy(out=exbv, in_=exv)
                    if g % 2 == 0:
                        tp2 = ps_tp.tile([81, 2, 128], BF16)
                    nc.tensor.transpose(tp2[:, g % 2, :], exb, identb)
                    if g % 2 == 1:
                        # pack both tiles of the pair with one copy:
                        # out (py, pair, px), in (py, pair, px)
                        opair = bass.AP(
                            tensor=oimg.tensor,
                            offset=oimg.offset + ty * BY * W + (tx - 1) * BX,
                            ap=[list(oimg.ap[0]), [W, BY], [BX, 2], [1, BX]],
                        )
                        nc.vector.tensor_copy(
                            out=opair,
                            in_=bass.AP(
                                tensor=tp2.tensor,
                                offset=tp2.offset,
                                ap=[list(tp2.ap[0]), [BX, BY],
                                    [128, 2], [1, BX]],
                            ),
                        )

            def finish_img(img, oimg):
                # rn1 applied on the assembled image, strip-wise row-major
                norm_f1(img)
                if mode == "noext":
                    nc.vector.memset(oimg, 0.0)
                else:
                    for s in range(8):
                        bct = ps_bc.tile([128, 512], FP32, tag="bc")
                        nc.tensor.matmul(
                            bct[0:81, :], lhsT=selr[s][:, 0:P2],
                            rhs=rn1bs[img], start=True, stop=True,
                        )
                        bcs = strip.tile([81, 512], BF16, tag="bcs81")
                        nc.scalar.copy(out=bcs, in_=bct[0:81, :])
                        osl = oimg[:, 512 * s : 512 * (s + 1)]
                        nc.vector.tensor_mul(osl, osl, bcs)
                for h in range(2):
                    nc.gpsimd.dma_start(
                        out=out[img, :, h * (HWTOT // 2) : (h + 1) * (HWTOT // 2)],
                        in_=oimg[:, h * (HWTOT // 2) : (h + 1) * (HWTOT // 2)],
                    )

            NB_ = NTY * NTX // GB  # 4 batches per image
            gds0, gds1 = [], []
            # img1's loads are emitted after img0's normalization so img0's
            # squares don't queue behind img1's restage copies on ACT/DVE
            load_img(0)
            norm_f2(0)
            load_img(1)
            for k in range(NB_):
                gram_batch(0, k, gds0)
            norm_f2(1)
            oimg0 = outpool.tile([81, HWTOT], BF16, tag="oimg")
            oimg1 = outpool.tile([81, HWTOT], BF16, tag="oimg")
            # interleave img1's Gram batches with img0's extraction batches:
            # img0's gathers/densify/packs overlap img1's matmuls instead of
            # queueing behind all of img1's relu work
            for k in range(NB_):
                gram_batch(1, k, gds1)
                if mode != "noext":
                    extract_batch(0, k, gds0, oimg0)
            finish_img(0, oimg0)

            def scale_strips(img, oimg, strips):
                for s in strips:
                    bct = ps_bc.tile([128, 512], FP32, tag="bc")
                    nc.tensor.matmul(
                        bct[0:81, :], lhsT=selr[s][:, 0:P2],
                        rhs=rn1bs[img], start=True, stop=True,
                    )
                    bcs = strip.tile([81, 512], BF16, tag="bcs81")
                    nc.scalar.copy(out=bcs, in_=bct[0:81, :])
                    osl = oimg[:, 512 * s : 512 * (s + 1)]
                    nc.vector.tensor_mul(osl, osl, bcs)

            if mode == "noext":
                finish_img(1, oimg1)
            else:
                # img1 tail software-pipelined: batch k covers image-row band
                # ty=k (strips 2k, 2k+1), so its rn1 scales and the output
                # halves fire as soon as their packs land instead of after
                # the whole image
                for k in range(NB_):
                    extract_batch(1, k, gds1, oimg1)
                    if k == 0:
                        norm_f1(1)
                    else:
                        scale_strips(1, oimg1, (2 * (k - 1), 2 * k - 1))
                    if k == 3:
                        nc.gpsimd.dma_start(
                            out=out[1, :, 0 : HWTOT // 2],
                            in_=oimg1[:, 0 : HWTOT // 2],
                        )
                scale_strips(1, oimg1, (6, 7))
                nc.gpsimd.dma_start(
                    out=out[1, :, HWTOT // 2 :],
                    in_=oimg1[:, HWTOT // 2 :],
                )

        for _rep in range(repeat):
            emit_once()
    return nc


# ---------------------------------------------------------------- host side
def _edge_masks():
    p = np.arange(128)
    d = np.arange(P2)
    px = (p % BX)[:, None]
    dw = (d % PATCH)[None, :]
    # tx = 0:      x_img = px + dw - R >= 0             <=>  px + dw >= R
    # tx = NTX-1:  x_img = (NTX-1)*BX + px + dw - R < W <=>  px + dw < BX + R
    m0 = (px + dw >= R).astype(np.float32)
    m7 = (px + dw < BX + R).astype(np.float32)
    return m0, m7


_cache = threading.local()


def _get_compiled():
    if getattr(_cache, "nc", None) is None:
        nc = bass.Bass()
        f1 = nc.dram_tensor("feature1", [B_LOC, C, H, W], FP32, kind="ExternalInput")
        f2 = nc.dram_tensor("feature2", [B_LOC, C, H, W], FP32, kind="ExternalInput")
        mask0 = nc.dram_tensor("mask0", [128, P2], FP32, kind="ExternalInput")
        mask7 = nc.dram_tensor("mask7", [128, P2], FP32, kind="ExternalInput")
        out = nc.dram_tensor("out", [B_LOC, P2, HWTOT], FP32, kind="ExternalOutput")
        build_matching_kernel(nc, f1.ap(), f2.ap(), mask0.ap(), mask7.ap(), out.ap())
        _split_sync_waits(nc, max_waits=1)
        _cache.nc = nc
    return _cache.nc


def kernel(feature1: np.ndarray, feature2: np.ndarray) -> np.ndarray:
    from concourse.bass_utils import run_bass_kernel_spmd

    feature1 = np.ascontiguousarray(feature1, dtype=np.float32)
    feature2 = np.ascontiguousarray(feature2, dtype=np.float32)
    nc = _get_compiled()
    m0, m7 = _edge_masks()
    rsel = _rowsel_np()
    in_maps = []
    for c in range(N_CORES):
        sl = slice(c * B_LOC, (c + 1) * B_LOC)
        in_maps.append(
            {
                "feature1": feature1[sl],
                "feature2": feature2[sl],
                "mask0": m0,
                "mask7": m7,
                "rowsel": rsel,
            }
        )
    res = run_bass_kernel_spmd(nc, in_maps, core_ids=list(range(N_CORES)))
    out = np.concatenate([res.results[c]["out"] for c in range(N_CORES)], axis=0)
    return out.reshape(B, P2, HWTOT)

